# revision 15
# baseline (speedup 1.0000x reference)
"""MultiHeadEMA on 8 Trainium2 NeuronCores.

Strategy
--------
Channel-sharded: embed_dim=1024 -> 8 slices of 128 channels (= SBUF
partitions), one per core. The reference's FFT conv is exactly an order-2 IIR
    y_n[l] = q_n y_n[l-1] + x[l],   out = silu(c0 y0 + c1 y1 + omega x)
computed with `tensor_tensor_scan` on the vector engine.

The DVE scan runs at ~2.1 cyc/elem, so the recurrence is decimated by 2:
    Y_n[m] = y_n[2m] satisfies  Y_n[m] = q_n^2 Y_n[m-1] + u_n[m]
    u_n[m] = x[2m] + q_n x[2m-1]
u_n is built by accumulating diagonal matmuls (tensor engine, bf16) into PSUM
from contiguous phase blocks of x (even / odd / odd-shifted, deinterleaved on
the host — strided matmul rhs halves PE throughput). The scan reads u straight
from PSUM at half length. Odd outputs are never materialized:
    pre_even = c0 Y0 + c1 Y1 + w x_e
    pre_odd  = (c0 q0) Y0 + (c1 q1) Y1 + (c0+c1+w) x_o
Even combines run as diagonal matmuls into PSUM; odd combines are split
between the tensor engine and the vector engine (bf16 tensor_scalar runs in
4x mode) to balance the two engines — with all 8 cores active the chip power
governor caps matmuls at ~379ns vs 216ns single-core, so PE work is the
binding resource. Silu evacuates PSUM (or SBUF) into a phase-major output
that the host re-interleaves. Interior is bf16 with fp32 PSUM accumulation,
fp32 scan state, and exact fp32 decay factors.
"""

import numpy as np
import ml_dtypes

import concourse.bass as bass
import concourse.bacc as bacc
import concourse.tile as tile
from concourse import mybir
from concourse.bass_utils import run_bass_kernel_spmd

SEQ_LEN, BSZ, EMBED_DIM, NDIM = 4096, 4, 1024, 2
N_CORES = 8
D_PER = EMBED_DIM // N_CORES  # 128 channels/core = full SBUF partitions
SCALE = (1.0 / NDIM) ** 0.5
M = SEQ_LEN // 2          # decimated length 2048
CH = 512                  # matmul/psum chunk (one fp32 PSUM bank)
NCH = M // CH             # 4 chunks per slab
UP = 1024                 # scan piece (2 PSUM banks)
# odd-combine chunks computed on DVE instead of PE (load balance)
DVE_ODD = (0, 2)
F32 = mybir.dt.float32
BF16 = mybir.dt.bfloat16
AF = mybir.ActivationFunctionType
ALU = mybir.AluOpType


def build_bass():
    nc = bacc.Bacc(name="multihead_ema")
    # x blocks: 0 = x[2m] (even), 1 = x[2m+1] (odd), 2 = x[2m-1] (odd shifted)
    x = nc.dram_tensor("x", [D_PER, BSZ, 3, M], BF16, kind="ExternalInput")
    # coef columns: [delta0, delta1, alpha0, alpha1, beta0, beta1, gamma0, gamma1, omega]
    coef = nc.dram_tensor("coef", [D_PER, 9], F32, kind="ExternalInput")
    eye = nc.dram_tensor("eye", [D_PER, D_PER], BF16, kind="ExternalInput")
    # out blocks: 0 = out[2m], 1 = out[2m+1]
    out = nc.dram_tensor("out", [D_PER, BSZ, 2, M], BF16, kind="ExternalOutput")

    with tile.TileContext(nc) as tc:
        with (
            tc.tile_pool(name="const", bufs=1) as const,
            tc.tile_pool(name="xp", bufs=3) as xp,
            tc.tile_pool(name="yp", bufs=2) as yp,
            tc.tile_pool(name="odde", bufs=4) as odde,
            tc.tile_pool(name="op", bufs=3) as op,
            tc.tile_pool(name="psu", bufs=2, space="PSUM") as psu,
            tc.tile_pool(name="psc", bufs=3, space="PSUM") as psc,
        ):
            csb = const.tile([D_PER, 9], F32)
            nc.sync.dma_start(out=csb[:, :], in_=coef[:, :])
            eyesb = const.tile([D_PER, D_PER], BF16)
            nc.sync.dma_start(out=eyesb[:, :], in_=eye[:, :])

            # --- per-channel coefficients ([128, 1/2] fp32, trivial)
            sig = const.tile([D_PER, 4], F32)  # [p0, p1, sa0, sa1]
            nc.scalar.activation(out=sig[:, :], in_=csb[:, 0:4], func=AF.Sigmoid)
            pq = const.tile([D_PER, NDIM], F32)
            nc.vector.tensor_mul(out=pq[:, :], in0=sig[:, 0:2], in1=sig[:, 2:4])
            q = const.tile([D_PER, NDIM], F32)  # q = 1 - p*sigmoid(alpha)
            nc.scalar.activation(out=q[:, :], in_=pq[:, :], func=AF.Copy,
                                 scale=-1.0, bias=1.0)
            q2 = const.tile([D_PER, NDIM], F32)
            nc.vector.tensor_mul(out=q2[:, :], in0=q[:, :], in1=q[:, :])
            c1t = const.tile([D_PER, NDIM], F32)
            nc.vector.tensor_mul(out=c1t[:, :], in0=sig[:, 0:2], in1=csb[:, 4:6])
            c2t = const.tile([D_PER, NDIM], F32)
            nc.vector.tensor_mul(out=c2t[:, :], in0=c1t[:, :], in1=csb[:, 6:8])
            cc = const.tile([D_PER, NDIM], F32)  # c_n = p beta gamma scale
            nc.scalar.mul(out=cc[:, :], in_=c2t[:, :], mul=SCALE)
            cq = const.tile([D_PER, NDIM], F32)  # c_n q_n
            nc.vector.tensor_mul(out=cq[:, :], in0=cc[:, :], in1=q[:, :])
            csum = const.tile([D_PER, 1], F32)   # c0 + c1 + w
            nc.vector.tensor_add(out=csum[:, :], in0=cc[:, 0:1], in1=cc[:, 1:2])
            nc.vector.tensor_add(out=csum[:, :], in0=csum[:, :], in1=csb[:, 8:9])

            # --- bf16 diagonal weight matrices
            _dn = [0]

            def diag(scalar_ap):
                _dn[0] += 1
                t = const.tile([D_PER, D_PER], BF16, tag=f"diag{_dn[0]}")
                nc.vector.tensor_scalar_mul(out=t[:, :], in0=eyesb[:, :],
                                            scalar1=scalar_ap)
                return t

            w_q = [diag(q[:, n : n + 1]) for n in range(NDIM)]
            w_c = [diag(cc[:, n : n + 1]) for n in range(NDIM)]
            w_cq = [diag(cq[:, n : n + 1]) for n in range(NDIM)]
            w_w = diag(csb[:, 8:9])
            w_cs = diag(csum[:, 0:1])

            q2b = [q2[:, n : n + 1].to_broadcast([D_PER, UP]) for n in range(NDIM)]

            for b in range(BSZ):
                xb = xp.tile([D_PER, 3, M], BF16)
                nc.sync.dma_start(out=xb[:, :, :], in_=x[:, b, :, :])

                # --- u_n in PSUM, Y_n = scan(q_n^2, u_n), piece-chained
                Y = []
                for n in range(NDIM):
                    yn = yp.tile([D_PER, M], BF16, tag=f"y{n}")
                    for p in range(M // UP):
                        pu = psu.tile([D_PER, UP], F32, tag="u")
                        for h in range(UP // CH):
                            s_m = bass.ts(p * (UP // CH) + h, CH)
                            sh = bass.ts(h, CH)
                            nc.tensor.matmul(pu[:, sh], eyesb[:, :], xb[:, 0, s_m],
                                             start=True, stop=False)
                            nc.tensor.matmul(pu[:, sh], w_q[n][:, :], xb[:, 2, s_m],
                                             start=False, stop=True)
                        init = 0.0 if p == 0 else yn[:, p * UP - 1 : p * UP]
                        nc.vector.tensor_tensor_scan(
                            out=yn[:, bass.ts(p, UP)], data0=q2b[n], data1=pu[:, :],
                            initial=init, op0=ALU.mult, op1=ALU.add,
                        )
                    Y.append(yn)

                # --- outputs
                ob = op.tile([D_PER, 2, M], BF16)
                for ci in range(NCH):
                    s = bass.ts(ci, CH)
                    # even: PE matmuls -> PSUM -> silu
                    pe = psc.tile([D_PER, CH], F32, tag="cmb")
                    nc.tensor.matmul(pe[:, :], w_c[0][:, :], Y[0][:, s], start=True, stop=False)
                    nc.tensor.matmul(pe[:, :], w_c[1][:, :], Y[1][:, s], start=False, stop=False)
                    nc.tensor.matmul(pe[:, :], w_w[:, :], xb[:, 0, s], start=False, stop=True)
                    nc.scalar.activation(out=ob[:, 0, s], in_=pe[:, :], func=AF.Silu)
                    if ci in DVE_ODD:
                        # odd on DVE: bf16 tensor_scalar (4x) + tensor_tensor (2x)
                        t0 = odde.tile([D_PER, CH], BF16, tag="t0")
                        nc.vector.tensor_scalar_mul(out=t0[:, :], in0=Y[0][:, s],
                                                    scalar1=cq[:, 0:1])
                        t1 = odde.tile([D_PER, CH], BF16, tag="t1")
                        nc.vector.tensor_scalar_mul(out=t1[:, :], in0=Y[1][:, s],
                                                    scalar1=cq[:, 1:2])
                        t2 = odde.tile([D_PER, CH], BF16, tag="t2")
                        nc.vector.tensor_scalar_mul(out=t2[:, :], in0=xb[:, 1, s],
                                                    scalar1=csum[:, 0:1])
                        nc.vector.tensor_add(out=t0[:, :], in0=t0[:, :], in1=t1[:, :])
                        nc.vector.tensor_add(out=t0[:, :], in0=t0[:, :], in1=t2[:, :])
                        nc.scalar.activation(out=ob[:, 1, s], in_=t0[:, :], func=AF.Silu)
                    else:
                        po = psc.tile([D_PER, CH], F32, tag="cmb")
                        nc.tensor.matmul(po[:, :], w_cq[0][:, :], Y[0][:, s], start=True, stop=False)
                        nc.tensor.matmul(po[:, :], w_cq[1][:, :], Y[1][:, s], start=False, stop=False)
                        nc.tensor.matmul(po[:, :], w_cs[:, :], xb[:, 1, s], start=False, stop=True)
                        nc.scalar.activation(out=ob[:, 1, s], in_=po[:, :], func=AF.Silu)
                nc.sync.dma_start(out=out[:, b, :, :], in_=ob[:, :, :])

    nc.compile()
    return nc


_CACHE: dict = {}


def _get_nc():
    if "nc" not in _CACHE:
        _CACHE["nc"] = build_bass()
    return _CACHE["nc"]


def make_in_maps(inputs):
    x = np.asarray(inputs["x"], np.float32)
    delta = np.asarray(inputs["delta"], np.float32).reshape(EMBED_DIM, NDIM)
    alpha = np.asarray(inputs["alpha"], np.float32).reshape(EMBED_DIM, NDIM)
    beta = np.asarray(inputs["beta"], np.float32).reshape(EMBED_DIM, NDIM)
    gamma = np.asarray(inputs["gamma"], np.float32).reshape(EMBED_DIM, NDIM)
    omega = np.asarray(inputs["omega"], np.float32).reshape(EMBED_DIM, 1)
    coef_full = np.concatenate([delta, alpha, beta, gamma, omega], axis=1)
    eye = np.eye(D_PER, dtype=ml_dtypes.bfloat16)
    in_maps = []
    for c in range(N_CORES):
        sl = slice(c * D_PER, (c + 1) * D_PER)
        xc = x[:, :, sl].transpose(2, 1, 0).astype(ml_dtypes.bfloat16)  # [128,B,L]
        ph = xc.reshape(D_PER, BSZ, M, 2).transpose(0, 1, 3, 2)  # [128,B,2,M]
        xph = np.zeros((D_PER, BSZ, 3, M), dtype=ml_dtypes.bfloat16)
        xph[:, :, 0:2] = ph
        xph[:, :, 2, 1:] = ph[:, :, 1, :-1]  # x[2m-1], leading zero
        in_maps.append(
            {"x": np.ascontiguousarray(xph),
             "coef": np.ascontiguousarray(coef_full[sl]), "eye": eye}
        )
    return in_maps


def gather_out(results):
    out = np.empty((SEQ_LEN, BSZ, EMBED_DIM), np.float32)
    for c in range(N_CORES):
        # [128, B, 2, M] phase-major -> [l = 2m+r, b, d]
        arr = results[c]["out"].astype(np.float32)
        out[:, :, c * D_PER : (c + 1) * D_PER] = arr.transpose(3, 2, 1, 0).reshape(
            SEQ_LEN, BSZ, D_PER
        )
    return out


def _run(inputs, **kwargs):
    nc = _get_nc()
    in_maps = make_in_maps(inputs)
    res = run_bass_kernel_spmd(nc, in_maps, core_ids=list(range(N_CORES)), **kwargs)
    return gather_out(res.results), res


def kernel(**inputs) -> np.ndarray:
    out, _ = _run(inputs)
    return out


# revision 16
# speedup vs baseline: 1.1534x; 1.1534x over previous
"""MultiHeadEMA on 8 Trainium2 NeuronCores.

Strategy
--------
Channel-sharded: embed_dim=1024 -> 8 slices of 128 channels (= SBUF
partitions), one per core. The reference's FFT conv is exactly an order-2 IIR
    y_n[l] = q_n y_n[l-1] + x[l],   out = silu(c0 y0 + c1 y1 + omega x)
computed with `tensor_tensor_scan` on the vector engine.

The DVE scan runs at ~2.1 cyc/elem, so the recurrence is decimated by 2:
    Y_n[m] = y_n[2m] satisfies  Y_n[m] = q_n^2 Y_n[m-1] + u_n[m]
    u_n[m] = x[2m] + q_n x[2m-1]
u_n is built by accumulating diagonal matmuls (tensor engine, bf16) into PSUM
from contiguous phase blocks of x (even / odd / odd-shifted, deinterleaved on
the host — strided matmul rhs halves PE throughput). The scan reads u straight
from PSUM at half length. Odd outputs are never materialized:
    pre_even = c0 Y0 + c1 Y1 + w x_e
    pre_odd  = (c0 q0) Y0 + (c1 q1) Y1 + (c0+c1+w) x_o
Even combines run as diagonal matmuls into PSUM; odd combines are split
between the tensor engine and the vector engine (bf16 tensor_scalar runs in
4x mode) to balance the two engines — with all 8 cores active the chip power
governor caps matmuls at ~379ns vs 216ns single-core, so PE work is the
binding resource. Silu evacuates PSUM (or SBUF) into a phase-major output
that the host re-interleaves. Interior is bf16 with fp32 PSUM accumulation,
fp32 scan state, and exact fp32 decay factors.
"""

import numpy as np
import ml_dtypes

import concourse.bass as bass
import concourse.bacc as bacc
import concourse.tile as tile
from concourse import mybir
from concourse.bass_utils import run_bass_kernel_spmd

SEQ_LEN, BSZ, EMBED_DIM, NDIM = 4096, 4, 1024, 2
N_CORES = 8
D_PER = EMBED_DIM // N_CORES  # 128 channels/core = full SBUF partitions
SCALE = (1.0 / NDIM) ** 0.5
M = SEQ_LEN // 2          # decimated length 2048
CH = 512                  # matmul/psum chunk (one fp32 PSUM bank)
NCH = M // CH             # 4 chunks per slab
UP = 1024                 # scan piece (2 PSUM banks)
# odd-combine chunks computed on DVE instead of PE (load balance)
DVE_ODD = ()
F32 = mybir.dt.float32
BF16 = mybir.dt.bfloat16
AF = mybir.ActivationFunctionType
ALU = mybir.AluOpType


def build_bass():
    nc = bacc.Bacc(name="multihead_ema")
    # x blocks: 0 = x[2m] (even), 1 = x[2m+1] (odd), 2 = x[2m-1] (odd shifted)
    x = nc.dram_tensor("x", [D_PER, BSZ, 3, M], BF16, kind="ExternalInput")
    # coef columns: [delta0, delta1, alpha0, alpha1, beta0, beta1, gamma0, gamma1, omega]
    coef = nc.dram_tensor("coef", [D_PER, 9], F32, kind="ExternalInput")
    eye = nc.dram_tensor("eye", [D_PER, D_PER], BF16, kind="ExternalInput")
    # out blocks: 0 = out[2m], 1 = out[2m+1]
    out = nc.dram_tensor("out", [D_PER, BSZ, 2, M], BF16, kind="ExternalOutput")

    with tile.TileContext(nc) as tc:
        with (
            tc.tile_pool(name="const", bufs=1) as const,
            tc.tile_pool(name="xp", bufs=3) as xp,
            tc.tile_pool(name="yp", bufs=2) as yp,
            tc.tile_pool(name="odde", bufs=4) as odde,
            tc.tile_pool(name="op", bufs=3) as op,
            tc.tile_pool(name="psu", bufs=2, space="PSUM") as psu,
            tc.tile_pool(name="psc", bufs=3, space="PSUM") as psc,
        ):
            csb = const.tile([D_PER, 9], F32)
            nc.sync.dma_start(out=csb[:, :], in_=coef[:, :])
            eyesb = const.tile([D_PER, D_PER], BF16)
            nc.sync.dma_start(out=eyesb[:, :], in_=eye[:, :])

            # --- per-channel coefficients ([128, 1/2] fp32, trivial)
            sig = const.tile([D_PER, 4], F32)  # [p0, p1, sa0, sa1]
            nc.scalar.activation(out=sig[:, :], in_=csb[:, 0:4], func=AF.Sigmoid)
            pq = const.tile([D_PER, NDIM], F32)
            nc.vector.tensor_mul(out=pq[:, :], in0=sig[:, 0:2], in1=sig[:, 2:4])
            q = const.tile([D_PER, NDIM], F32)  # q = 1 - p*sigmoid(alpha)
            nc.scalar.activation(out=q[:, :], in_=pq[:, :], func=AF.Copy,
                                 scale=-1.0, bias=1.0)
            q2 = const.tile([D_PER, NDIM], F32)
            nc.vector.tensor_mul(out=q2[:, :], in0=q[:, :], in1=q[:, :])
            c1t = const.tile([D_PER, NDIM], F32)
            nc.vector.tensor_mul(out=c1t[:, :], in0=sig[:, 0:2], in1=csb[:, 4:6])
            c2t = const.tile([D_PER, NDIM], F32)
            nc.vector.tensor_mul(out=c2t[:, :], in0=c1t[:, :], in1=csb[:, 6:8])
            cc = const.tile([D_PER, NDIM], F32)  # c_n = p beta gamma scale
            nc.scalar.mul(out=cc[:, :], in_=c2t[:, :], mul=SCALE)
            cq = const.tile([D_PER, NDIM], F32)  # c_n q_n
            nc.vector.tensor_mul(out=cq[:, :], in0=cc[:, :], in1=q[:, :])
            csum = const.tile([D_PER, 1], F32)   # c0 + c1 + w
            nc.vector.tensor_add(out=csum[:, :], in0=cc[:, 0:1], in1=cc[:, 1:2])
            nc.vector.tensor_add(out=csum[:, :], in0=csum[:, :], in1=csb[:, 8:9])

            # --- bf16 diagonal weight matrices
            _dn = [0]

            def diag(scalar_ap):
                _dn[0] += 1
                t = const.tile([D_PER, D_PER], BF16, tag=f"diag{_dn[0]}")
                nc.vector.tensor_scalar_mul(out=t[:, :], in0=eyesb[:, :],
                                            scalar1=scalar_ap)
                return t

            w_q = [diag(q[:, n : n + 1]) for n in range(NDIM)]
            w_c = [diag(cc[:, n : n + 1]) for n in range(NDIM)]
            w_cq = [diag(cq[:, n : n + 1]) for n in range(NDIM)]
            w_w = diag(csb[:, 8:9])
            w_cs = diag(csum[:, 0:1])

            q2b = [q2[:, n : n + 1].to_broadcast([D_PER, UP]) for n in range(NDIM)]

            for b in range(BSZ):
                xb = xp.tile([D_PER, 3, M], BF16)
                nc.sync.dma_start(out=xb[:, :, :], in_=x[:, b, :, :])

                # --- u_n in PSUM, Y_n = scan(q_n^2, u_n), piece-chained
                Y = []
                for n in range(NDIM):
                    yn = yp.tile([D_PER, M], BF16, tag=f"y{n}")
                    for p in range(M // UP):
                        pu = psu.tile([D_PER, UP], F32, tag="u")
                        for h in range(UP // CH):
                            s_m = bass.ts(p * (UP // CH) + h, CH)
                            sh = bass.ts(h, CH)
                            nc.tensor.matmul(pu[:, sh], eyesb[:, :], xb[:, 0, s_m],
                                             start=True, stop=False)
                            nc.tensor.matmul(pu[:, sh], w_q[n][:, :], xb[:, 2, s_m],
                                             start=False, stop=True)
                        init = 0.0 if p == 0 else yn[:, p * UP - 1 : p * UP]
                        nc.vector.tensor_tensor_scan(
                            out=yn[:, bass.ts(p, UP)], data0=q2b[n], data1=pu[:, :],
                            initial=init, op0=ALU.mult, op1=ALU.add,
                        )
                    Y.append(yn)

                # --- outputs
                ob = op.tile([D_PER, 2, M], BF16)
                for ci in range(NCH):
                    s = bass.ts(ci, CH)
                    # even: PE matmuls -> PSUM -> silu
                    pe = psc.tile([D_PER, CH], F32, tag="cmb")
                    nc.tensor.matmul(pe[:, :], w_c[0][:, :], Y[0][:, s], start=True, stop=False)
                    nc.tensor.matmul(pe[:, :], w_c[1][:, :], Y[1][:, s], start=False, stop=False)
                    nc.tensor.matmul(pe[:, :], w_w[:, :], xb[:, 0, s], start=False, stop=True)
                    nc.scalar.activation(out=ob[:, 0, s], in_=pe[:, :], func=AF.Silu)
                    if ci in DVE_ODD:
                        # odd on DVE: bf16 tensor_scalar (4x) + tensor_tensor (2x)
                        t0 = odde.tile([D_PER, CH], BF16, tag="t0")
                        nc.vector.tensor_scalar_mul(out=t0[:, :], in0=Y[0][:, s],
                                                    scalar1=cq[:, 0:1])
                        t1 = odde.tile([D_PER, CH], BF16, tag="t1")
                        nc.vector.tensor_scalar_mul(out=t1[:, :], in0=Y[1][:, s],
                                                    scalar1=cq[:, 1:2])
                        t2 = odde.tile([D_PER, CH], BF16, tag="t2")
                        nc.vector.tensor_scalar_mul(out=t2[:, :], in0=xb[:, 1, s],
                                                    scalar1=csum[:, 0:1])
                        nc.vector.tensor_add(out=t0[:, :], in0=t0[:, :], in1=t1[:, :])
                        nc.vector.tensor_add(out=t0[:, :], in0=t0[:, :], in1=t2[:, :])
                        nc.scalar.activation(out=ob[:, 1, s], in_=t0[:, :], func=AF.Silu)
                    else:
                        po = psc.tile([D_PER, CH], F32, tag="cmb")
                        nc.tensor.matmul(po[:, :], w_cq[0][:, :], Y[0][:, s], start=True, stop=False)
                        nc.tensor.matmul(po[:, :], w_cq[1][:, :], Y[1][:, s], start=False, stop=False)
                        nc.tensor.matmul(po[:, :], w_cs[:, :], xb[:, 1, s], start=False, stop=True)
                        nc.scalar.activation(out=ob[:, 1, s], in_=po[:, :], func=AF.Silu)
                nc.sync.dma_start(out=out[:, b, :, :], in_=ob[:, :, :])

    nc.compile()
    return nc


_CACHE: dict = {}


def _get_nc():
    if "nc" not in _CACHE:
        _CACHE["nc"] = build_bass()
    return _CACHE["nc"]


def make_in_maps(inputs):
    x = np.asarray(inputs["x"], np.float32)
    delta = np.asarray(inputs["delta"], np.float32).reshape(EMBED_DIM, NDIM)
    alpha = np.asarray(inputs["alpha"], np.float32).reshape(EMBED_DIM, NDIM)
    beta = np.asarray(inputs["beta"], np.float32).reshape(EMBED_DIM, NDIM)
    gamma = np.asarray(inputs["gamma"], np.float32).reshape(EMBED_DIM, NDIM)
    omega = np.asarray(inputs["omega"], np.float32).reshape(EMBED_DIM, 1)
    coef_full = np.concatenate([delta, alpha, beta, gamma, omega], axis=1)
    eye = np.eye(D_PER, dtype=ml_dtypes.bfloat16)
    in_maps = []
    for c in range(N_CORES):
        sl = slice(c * D_PER, (c + 1) * D_PER)
        xc = x[:, :, sl].transpose(2, 1, 0).astype(ml_dtypes.bfloat16)  # [128,B,L]
        ph = xc.reshape(D_PER, BSZ, M, 2).transpose(0, 1, 3, 2)  # [128,B,2,M]
        xph = np.zeros((D_PER, BSZ, 3, M), dtype=ml_dtypes.bfloat16)
        xph[:, :, 0:2] = ph
        xph[:, :, 2, 1:] = ph[:, :, 1, :-1]  # x[2m-1], leading zero
        in_maps.append(
            {"x": np.ascontiguousarray(xph),
             "coef": np.ascontiguousarray(coef_full[sl]), "eye": eye}
        )
    return in_maps


def gather_out(results):
    out = np.empty((SEQ_LEN, BSZ, EMBED_DIM), np.float32)
    for c in range(N_CORES):
        # [128, B, 2, M] phase-major -> [l = 2m+r, b, d]
        arr = results[c]["out"].astype(np.float32)
        out[:, :, c * D_PER : (c + 1) * D_PER] = arr.transpose(3, 2, 1, 0).reshape(
            SEQ_LEN, BSZ, D_PER
        )
    return out


def _run(inputs, **kwargs):
    nc = _get_nc()
    in_maps = make_in_maps(inputs)
    res = run_bass_kernel_spmd(nc, in_maps, core_ids=list(range(N_CORES)), **kwargs)
    return gather_out(res.results), res


def kernel(**inputs) -> np.ndarray:
    out, _ = _run(inputs)
    return out
